# revision 3
# baseline (speedup 1.0000x reference)
"""Trainium2 Bass kernel for the autoregressive LSTM — fp8 DoubleRow body
with a bf16 tail.

Math (Keras LSTMCell, gate order i,f,g,o in the reference):
    z = x @ Wk + h @ Wr + b
    c = sigmoid(f)*c + sigmoid(i)*tanh(g)
    h = sigmoid(o)*tanh(c)
Warmup over T=256 input steps, then S=64 autoregressive decode steps through
a dense head p = h @ Wd + bd fed back as the next input.

Sharding: pure data-parallel over batch, 1024/8 = 128 examples per core.

Precision schedule: the first T-NTAIL warmup steps run with h cast to
fp8e4m3 (x8) and Wr prepacked fp8 (x32) in DoubleRow pair layout — one
DoubleRow matmul contracts TWO 128-row unit chunks. LSTM forget-gate damping
(~0.5/step) erases that quantization noise within ~20 steps, so the last
NTAIL warmup steps and the whole decode run in bf16 (baseline numerics);
outputs match the bf16 kernel. PSUM holds 256*z uniformly in both modes
(Wk, bf16-Wr, bf16-Wd all pre-scaled x256); gate ACTs apply scale 1/256.

fp8 step epilogue: h8 = (sig(o)*8)*tanh(c) via one fused DVE op, then a
u16-pair xbar transpose + engine-alternating deinterleave copies produce the
contiguous [K, 2, M] DoubleRow stationary tile. bf16 step epilogue: hbf =
sig(o)*tanh(c) bf16 + merged xbar transpose into chunk-major [128, NU, 128].
Transposes alternate between the SP and ACT DMA queues so the 4 per step run
on two rings.
"""

import sys

sys.path.insert(0, "/opt/trn_rl_repo")

import numpy as np

import concourse.bass as bass
import concourse.bacc as bacc
import concourse.mybir as mybir
from concourse.tile import TileContext
from concourse.bass_utils import run_bass_kernel_spmd

F32 = mybir.dt.float32
BF16 = mybir.dt.bfloat16
F8 = mybir.dt.float8e4
U16 = mybir.dt.uint16
NPBF16 = mybir.dt.np(mybir.dt.bfloat16)
NPF8 = mybir.dt.np(mybir.dt.float8e4)
AF = mybir.ActivationFunctionType
ALU = mybir.AluOpType
DR = mybir.MatmulPerfMode.DoubleRow

B, T, I, U, S = 1024, 256, 64, 1024, 64
NCORES = 8
BC = B // NCORES          # 128 batch per core
KX = I + 1                # x rows + ones row for folded bias
XBLK = 4                  # warmup steps per input-stream DMA block
NU = U // 128             # 8 recurrent k-chunks (bf16 path)

NW = 4                    # waves per step (each covers U/NW units, 4U/NW z-cols)
QW = U // NW              # units per wave (256)
WW = 4 * QW               # z columns per wave (1024)
NB = WW // 512            # 512-col bank slices per wave (2)
NG = NB                   # 512-col DR groups per wave (2): one per bank

SH = 8.0                  # h scale into fp8
SW = 32.0                 # weight scale into fp8
SZ = SH * SW              # PSUM holds SZ * z in BOTH modes
NTAIL = 8                 # final warmup steps in bf16 (decode is always bf16)


def _gate_perm():
    """Column permutation: reference gate order [i|f|g|o] (1024 each) ->
    NW waves of [i_q | f_q | o_q | g_q] (QW each)."""
    i0, f0, g0, o0 = 0, U, 2 * U, 3 * U
    parts = []
    for w in range(NW):
        for g in (i0, f0, o0, g0):
            parts.append(np.arange(QW) + g + w * QW)
    return np.concatenate(parts)


def build_nc(n_warm=T, n_dec=S - 1, n_tail=NTAIL, for_sim=False):
    nc = bacc.Bacc(None, target_bir_lowering=False, debug=True) if for_sim else bacc.Bacc()
    n_fp8 = max(0, n_warm - n_tail)

    nblk = (n_warm + XBLK - 1) // XBLK
    xTbD = nc.declare_dram_parameter("xTb", [nblk, KX, XBLK * BC], BF16, isOutput=False)
    WkD = nc.declare_dram_parameter("Wk", [KX, 4 * U], BF16, isOutput=False)
    # Wr8[k, p, g, i, n] = SW * Wr_perm[256p + 2k + i, 512g + n]
    WrD = nc.declare_dram_parameter("Wr8", [128, NW, 8, 2, 512], F8, isOutput=False)
    WrbD = nc.declare_dram_parameter("Wrb", [128, NU, 4 * U], BF16, isOutput=False)
    WdbD = nc.declare_dram_parameter("Wdb", [128, NU, I], BF16, isOutput=False)
    bdD = nc.declare_dram_parameter("bdc", [I, 1], F32, isOutput=False)
    outD = nc.declare_dram_parameter("out", [n_dec + 1, I, BC], F32, isOutput=True)

    with TileContext(nc) as tc:
        with (
            tc.tile_pool(name="const", bufs=1) as cpool,
            tc.tile_pool(name="xp", bufs=2) as xpool,
            tc.tile_pool(name="state", bufs=2) as hpool,
            tc.tile_pool(name="gates", bufs=2) as gpool,
            tc.tile_pool(name="psum", bufs=3, space="PSUM") as zpool,
        ):
            Wk_sb = cpool.tile([KX, 4 * U], BF16)
            Wr_sb = cpool.tile([128, NW, 8, 2, 512], F8)
            Wrb_sb = cpool.tile([128, NU, 4 * U], BF16)
            Wdb_sb = cpool.tile([128, NU, I], BF16)
            bd_sb = cpool.tile([I, 1], F32)
            c_sb = cpool.tile([128, U], F32)
            nc.sync.dma_start(Wk_sb[:], WkD[:])
            if n_fp8 > 0:
                nc.sync.dma_start(Wr_sb[:], WrD[:])
            nc.sync.dma_start(Wrb_sb[:], WrbD[:])
            nc.sync.dma_start(Wdb_sb[:], WdbD[:])
            nc.sync.dma_start(bd_sb[:], bdD[:])
            nc.gpsimd.memset(c_sb[:], 0.0)

            def emit_step(x_lhsT, hT_prev, hTb_prev, x_first, out_fmt):
                """One LSTM step. Consumes EITHER hT_prev (fp8 pair tiles)
                or hTb_prev (bf16 chunk-major tile). Produces hT (list) or
                hTb per out_fmt."""
                use_fp8 = hT_prev is not None
                have_h = use_fp8 or hTb_prev is not None
                hT_new = [None] * NW
                hTb_new = (
                    hpool.tile([128, NU, 128], BF16, tag="hTb", name="hTb")
                    if out_fmt == "bf16"
                    else None
                )
                for w in range(NW):
                    z = zpool.tile([128, WW], F32, tag="z", name="z")
                    # Ordered op list; PSUM zero regions are bank-granular
                    # (2KB): exactly one start (first writer) and one stop
                    # (last writer) per bank.
                    ops = []  # (kind, idx, g, bank)
                    if x_first and x_lhsT is not None:
                        ops += [("x", nb, None, nb) for nb in range(NB)]
                    if use_fp8:
                        ops += [
                            ("h8", p, g, g)
                            for p in range(NW)
                            for g in range(NG)
                        ]
                    elif have_h:
                        ops += [
                            ("hb", u, nb, nb)
                            for u in range(NU)
                            for nb in range(NB)
                        ]
                    if not x_first and x_lhsT is not None:
                        ops += [("x", nb, None, nb) for nb in range(NB)]
                    first_by_bank, last_by_bank = {}, {}
                    for i, op in enumerate(ops):
                        first_by_bank.setdefault(op[3], i)
                        last_by_bank[op[3]] = i
                    for i, (kind, idx, g, bank) in enumerate(ops):
                        start = i == first_by_bank[bank]
                        stop = i == last_by_bank[bank]
                        if kind == "x":
                            nc.tensor.matmul(
                                z[:, idx * 512 : (idx + 1) * 512],
                                x_lhsT,
                                Wk_sb[:, WW * w + idx * 512 : WW * w + (idx + 1) * 512],
                                start=start,
                                stop=stop,
                            )
                        elif kind == "h8":
                            nc.tensor.matmul(
                                z[:, g * 512 : (g + 1) * 512],
                                hT_prev[idx][:],
                                Wr_sb[:, idx, NG * w + g],
                                start=start,
                                stop=stop,
                                perf_mode=DR,
                            )
                        else:  # hb: bf16 chunk idx, bank g
                            nc.tensor.matmul(
                                z[:, g * 512 : (g + 1) * 512],
                                hTb_prev[:, idx, :],
                                Wrb_sb[:, idx, WW * w + g * 512 : WW * w + (g + 1) * 512],
                                start=start,
                                stop=stop,
                            )
                    # Gate math. Wave layout [i|f|o|g], each QW wide; PSUM
                    # holds SZ*z so every z-reading ACT applies scale 1/SZ.
                    # One merged sigmoid over the contiguous [i|f|o] block.
                    sig = gpool.tile([128, 3 * QW], F32, tag="sig", name="sig")
                    tg = gpool.tile([128, QW], F32, tag="tg", name="tg")
                    nc.scalar.activation(sig[:], z[:, 0 : 3 * QW], AF.Sigmoid, scale=1.0 / SZ)
                    nc.scalar.activation(tg[:], z[:, 3 * QW : 4 * QW], AF.Tanh, scale=1.0 / SZ)
                    cs = c_sb[:, w * QW : (w + 1) * QW]
                    t1 = gpool.tile([128, QW], F32, tag="t1", name="t1")
                    # t1 = sig(i)*tanh(g) on Pool (mostly idle engine)
                    nc.gpsimd.tensor_mul(t1[:], sig[:, 0:QW], tg[:])
                    t2 = gpool.tile([128, QW], F32, tag="t2", name="t2")
                    nc.vector.tensor_mul(t2[:], sig[:, QW : 2 * QW], cs)
                    nc.vector.tensor_add(cs, t1[:], t2[:])
                    tcc = gpool.tile([128, QW], F32, tag="tcc", name="tcc")
                    nc.scalar.activation(tcc[:], cs, AF.Tanh)
                    if out_fmt == "fp8":
                        # h8 = (sig(o) * SH) * tanh(c), fp8, one fused DVE op
                        h8 = gpool.tile([128, QW], F8, tag="h8", name="h8")
                        nc.vector.scalar_tensor_tensor(
                            h8[:], sig[:, 2 * QW : 3 * QW], SH, tcc[:],
                            ALU.mult, ALU.mult,
                        )
                        # u16-pair xbar transpose: word k of line m holds
                        # units (2k, 2k+1); deinterleave to the contiguous
                        # [K, 2, M] DoubleRow stationary layout (A = even
                        # units of the wave, B = odd).
                        hT16 = hpool.tile([128, QW // 2], U16, tag=f"hS{w}", name="hS")
                        nc.sync.dma_start_transpose(hT16[:], h8[:].bitcast(U16))
                        hTd = hpool.tile([128, 2, 128], F8, tag=f"hT{w}", name="hT")
                        hv = hT16[:].bitcast(F8).rearrange("k (m two) -> k two m", two=2)
                        nc.vector.tensor_copy(hTd[:, 0, :], hv[:, 0, :])
                        nc.vector.tensor_copy(hTd[:, 1, :], hv[:, 1, :])
                        hT_new[w] = hTd
                    else:
                        hbf = gpool.tile([128, QW], BF16, tag="hbf", name="hbf")
                        nc.vector.tensor_mul(hbf[:], sig[:, 2 * QW : 3 * QW], tcc[:])
                        nch = QW // 128
                        nc.sync.dma_start_transpose(
                            hTb_new[:, w * nch : (w + 1) * nch, :], hbf[:]
                        )
                return (hT_new, hTb_new)

            def emit_dense(hTb_cur, out_idx, feedback):
                zp = zpool.tile([128, WW], F32, tag="z", name="zdense")
                zd = zp[0:I, 0:BC]
                for u in range(NU):
                    nc.tensor.matmul(
                        zd,
                        Wdb_sb[:, u, :],
                        hTb_cur[:, u, :],
                        start=(u == 0),
                        stop=(u == NU - 1),
                    )
                pf = gpool.tile([I, BC], F32, tag="pf", name="pf")
                nc.scalar.activation(pf[:], zd, AF.Identity, bias=bd_sb[:], scale=1.0 / SZ)
                nc.scalar.dma_start(outD[out_idx], pf[:])
                if not feedback:
                    return None
                pt = gpool.tile([KX, BC], BF16, tag="pT", name="pT")
                nc.gpsimd.memset(pt[I : I + 1, :], 1.0)
                nc.scalar.activation(pt[0:I, :], zd, AF.Identity, bias=bd_sb[:], scale=1.0 / SZ)
                return pt

            hT, hTb = None, None
            nblk_used = (n_warm + XBLK - 1) // XBLK
            xtiles = {}
            if nblk_used > 0:
                xtiles[0] = xpool.tile([KX, XBLK * BC], BF16, tag="xblk", name="xblk")
                nc.sync.dma_start(xtiles[0][:], xTbD[0])
            for t in range(n_warm):
                b = t // XBLK
                s = t % XBLK
                fmt = "fp8" if t < n_fp8 - 1 else "bf16"
                hT, hTb = emit_step(
                    xtiles[b][:, s * BC : (s + 1) * BC], hT, hTb,
                    x_first=True, out_fmt=fmt,
                )
                if fmt == "bf16":
                    hT = None
                if t % XBLK == 0 and b + 1 < nblk_used:
                    xtiles[b + 1] = xpool.tile([KX, XBLK * BC], BF16, tag="xblk", name="xblk")
                    nc.sync.dma_start(xtiles[b + 1][:], xTbD[b + 1])
                xtiles.pop(b - 1, None)
            pt = emit_dense(hTb, 0, feedback=(n_dec > 0))
            for d in range(n_dec):
                _, hTb = emit_step(
                    pt[:] if pt is not None else None, None, hTb,
                    x_first=False, out_fmt="bf16",
                )
                pt = emit_dense(hTb, d + 1, feedback=(d < n_dec - 1))

    nc.finalize()
    return nc


def prep_in_maps(inputs, Wk, Wr, b, Wd, bd, n_warm=T):
    """Host-side sharding + layout. inputs [B, T, I] fp32; returns 8 in_maps."""
    perm = _gate_perm()
    Wk_aug = np.concatenate(
        [np.asarray(Wk, np.float32), np.asarray(b, np.float32)[None, :]], axis=0
    )
    Wk_p = (Wk_aug[:, perm] * SZ).astype(NPBF16)               # [65, 4096], x256
    Wr_p = np.asarray(Wr, np.float32)[:, perm]                 # [1024, 4096]
    # Wr8[k, p, g, i, n] = SW * Wr_p[256p + 2k + i, 512g + n]
    Wr8 = (Wr_p * SW).astype(NPF8)
    Wr8 = Wr8.reshape(NW, 128, 2, 8, 512)                      # [p, k, i, g, n]
    Wr8 = Wr8.transpose(1, 0, 3, 2, 4).copy()                  # [k, p, g, i, n]
    # bf16 tail weights, same x256 z-scale as the fp8 path
    Wrb = (Wr_p * SZ).astype(NPBF16)
    Wrb = Wrb.reshape(NU, 128, 4 * U).transpose(1, 0, 2).copy()
    Wdb = (np.asarray(Wd, np.float32) * SZ).astype(NPBF16)
    Wdb = Wdb.reshape(NU, 128, I).transpose(1, 0, 2).copy()
    bd_c = np.asarray(bd, np.float32).reshape(I, 1).copy()

    x = np.asarray(inputs, np.float32)
    nblk = (n_warm + XBLK - 1) // XBLK
    in_maps = []
    for c in range(NCORES):
        xc = x[c * BC : (c + 1) * BC, :n_warm]                 # [BC, n_warm, I]
        xT = np.transpose(xc, (1, 2, 0))                       # [n_warm, I, BC]
        xTa = np.concatenate([xT, np.ones((n_warm, 1, BC), np.float32)], axis=1)
        if nblk * XBLK != n_warm:
            pad = np.zeros((nblk * XBLK - n_warm, KX, BC), np.float32)
            xTa = np.concatenate([xTa, pad], axis=0)
        xTb = (
            xTa.reshape(nblk, XBLK, KX, BC)
            .transpose(0, 2, 1, 3)
            .reshape(nblk, KX, XBLK * BC)
            .astype(NPBF16)
            .copy()
        )
        in_maps.append(
            {"xTb": xTb, "Wk": Wk_p, "Wr8": Wr8, "Wrb": Wrb, "Wdb": Wdb, "bdc": bd_c}
        )
    return in_maps


_NC_CACHE = {}


def _get_nc(n_warm, n_dec):
    key = (n_warm, n_dec)
    if key not in _NC_CACHE:
        _NC_CACHE[key] = build_nc(n_warm, n_dec)
    return _NC_CACHE[key]


def run(inputs, Wk, Wr, b, Wd, bd, n_warm, n_dec, trace=False):
    nc = _get_nc(n_warm, n_dec)
    in_maps = prep_in_maps(inputs, Wk, Wr, b, Wd, bd, n_warm)
    res = run_bass_kernel_spmd(nc, in_maps, list(range(NCORES)), trace=trace)
    outs = [np.asarray(res.results[c]["out"], np.float32) for c in range(NCORES)]
    # out[c]: [n_dec+1, I, BC] -> preds [B, n_dec+1, I]
    preds = np.concatenate([o.transpose(2, 0, 1) for o in outs], axis=0)
    return preds, res


def kernel(inputs, Wk, Wr, b, Wd, bd, output_indices, output_steps):
    n_dec = int(output_steps) - 1
    preds, _ = run(inputs, Wk, Wr, b, Wd, bd, T, n_dec)
    idx = np.asarray(output_indices, np.int64)
    return np.take(preds, idx, axis=-1).astype(np.float32)


# revision 4
# speedup vs baseline: 1.0636x; 1.0636x over previous
"""Trainium2 Bass kernel for the autoregressive LSTM — fp8 DoubleRow body
with a bf16 tail.

Math (Keras LSTMCell, gate order i,f,g,o in the reference):
    z = x @ Wk + h @ Wr + b
    c = sigmoid(f)*c + sigmoid(i)*tanh(g)
    h = sigmoid(o)*tanh(c)
Warmup over T=256 input steps, then S=64 autoregressive decode steps through
a dense head p = h @ Wd + bd fed back as the next input.

Sharding: pure data-parallel over batch, 1024/8 = 128 examples per core.

Precision schedule: the first T-NTAIL warmup steps run with h cast to
fp8e4m3 (x8) and Wr prepacked fp8 (x32) in DoubleRow pair layout — one
DoubleRow matmul contracts TWO 128-row unit chunks. LSTM forget-gate damping
(~0.5/step) erases that quantization noise within ~20 steps, so the last
NTAIL warmup steps and the whole decode run in bf16 (baseline numerics);
outputs match the bf16 kernel. PSUM holds 256*z uniformly in both modes
(Wk, bf16-Wr, bf16-Wd all pre-scaled x256); gate ACTs apply scale 1/256.

fp8 step epilogue: h8 = (sig(o)*8)*tanh(c) via one fused DVE op, then a
u16-pair xbar transpose + engine-alternating deinterleave copies produce the
contiguous [K, 2, M] DoubleRow stationary tile. bf16 step epilogue: hbf =
sig(o)*tanh(c) bf16 + merged xbar transpose into chunk-major [128, NU, 128].
Transposes alternate between the SP and ACT DMA queues so the 4 per step run
on two rings.
"""

import sys

sys.path.insert(0, "/opt/trn_rl_repo")

import numpy as np

import concourse.bass as bass
import concourse.bacc as bacc
import concourse.mybir as mybir
from concourse.tile import TileContext
from concourse.bass_utils import run_bass_kernel_spmd

F32 = mybir.dt.float32
BF16 = mybir.dt.bfloat16
F8 = mybir.dt.float8e4
U16 = mybir.dt.uint16
NPBF16 = mybir.dt.np(mybir.dt.bfloat16)
NPF8 = mybir.dt.np(mybir.dt.float8e4)
AF = mybir.ActivationFunctionType
ALU = mybir.AluOpType
DR = mybir.MatmulPerfMode.DoubleRow

B, T, I, U, S = 1024, 256, 64, 1024, 64
NCORES = 8
BC = B // NCORES          # 128 batch per core
KX = I + 1                # x rows + ones row for folded bias
XBLK = 4                  # warmup steps per input-stream DMA block
NU = U // 128             # 8 recurrent k-chunks (bf16 path)

NW = 4                    # waves per step (each covers U/NW units, 4U/NW z-cols)
QW = U // NW              # units per wave (256)
WW = 4 * QW               # z columns per wave (1024)
NB = WW // 512            # 512-col bank slices per wave (2)
NG = NB                   # 512-col DR groups per wave (2): one per bank

SH = 8.0                  # h scale into fp8
SW = 32.0                 # weight scale into fp8
SZ = SH * SW              # PSUM holds SZ * z in BOTH modes
NTAIL = 8                 # final warmup steps in bf16 (decode is always bf16)


def _gate_perm():
    """Column permutation: reference gate order [i|f|g|o] (1024 each) ->
    NW waves of [i_q | f_q | o_q | g_q] (QW each)."""
    i0, f0, g0, o0 = 0, U, 2 * U, 3 * U
    parts = []
    for w in range(NW):
        for g in (i0, f0, o0, g0):
            parts.append(np.arange(QW) + g + w * QW)
    return np.concatenate(parts)


def build_nc(n_warm=T, n_dec=S - 1, n_tail=NTAIL, for_sim=False):
    nc = bacc.Bacc(None, target_bir_lowering=False, debug=True) if for_sim else bacc.Bacc()
    n_fp8 = max(0, n_warm - n_tail)

    nblk = (n_warm + XBLK - 1) // XBLK
    xTbD = nc.declare_dram_parameter("xTb", [nblk, KX, XBLK * BC], BF16, isOutput=False)
    WkD = nc.declare_dram_parameter("Wk", [KX, 4 * U], BF16, isOutput=False)
    # Wr8[k, p, g, i, n] = SW * Wr_perm[256p + 2k + i, 512g + n]
    WrD = nc.declare_dram_parameter("Wr8", [128, NW, 8, 2, 512], F8, isOutput=False)
    WrbD = nc.declare_dram_parameter("Wrb", [128, NU, 4 * U], BF16, isOutput=False)
    WdbD = nc.declare_dram_parameter("Wdb", [128, NU, I], BF16, isOutput=False)
    bdD = nc.declare_dram_parameter("bdc", [I, 1], F32, isOutput=False)
    outD = nc.declare_dram_parameter("out", [n_dec + 1, I, BC], F32, isOutput=True)

    with TileContext(nc) as tc:
        with (
            tc.tile_pool(name="const", bufs=1) as cpool,
            tc.tile_pool(name="xp", bufs=2) as xpool,
            tc.tile_pool(name="state", bufs=2) as hpool,
            tc.tile_pool(name="gates", bufs=2) as gpool,
            tc.tile_pool(name="psum", bufs=3, space="PSUM") as zpool,
        ):
            Wk_sb = cpool.tile([KX, 4 * U], BF16)
            Wr_sb = cpool.tile([128, NW, 8, 2, 512], F8)
            Wrb_sb = cpool.tile([128, NU, 4 * U], BF16)
            Wdb_sb = cpool.tile([128, NU, I], BF16)
            bd_sb = cpool.tile([I, 1], F32)
            c_sb = cpool.tile([128, U], F32)
            nc.sync.dma_start(Wk_sb[:], WkD[:])
            if n_fp8 > 0:
                nc.sync.dma_start(Wr_sb[:], WrD[:])
            nc.sync.dma_start(Wrb_sb[:], WrbD[:])
            nc.sync.dma_start(Wdb_sb[:], WdbD[:])
            nc.sync.dma_start(bd_sb[:], bdD[:])
            nc.gpsimd.memset(c_sb[:], 0.0)

            def emit_step(x_lhsT, hT_prev, hTb_prev, x_first, out_fmt):
                """One LSTM step. Consumes EITHER hT_prev (fp8 pair tiles)
                or hTb_prev (bf16 chunk-major tile). Produces hT (list) or
                hTb per out_fmt."""
                use_fp8 = hT_prev is not None
                have_h = use_fp8 or hTb_prev is not None
                hT_new = [None] * NW
                hTb_new = (
                    hpool.tile([128, NU, 128], BF16, tag="hTb", name="hTb")
                    if out_fmt == "bf16"
                    else None
                )
                for w in range(NW):
                    z = zpool.tile([128, WW], F32, tag="z", name="z")
                    # Ordered op list; PSUM zero regions are bank-granular
                    # (2KB): exactly one start (first writer) and one stop
                    # (last writer) per bank.
                    ops = []  # (kind, idx, g, bank)
                    if x_first and x_lhsT is not None:
                        ops += [("x", nb, None, nb) for nb in range(NB)]
                    if use_fp8:
                        ops += [
                            ("h8", p, g, g)
                            for p in range(NW)
                            for g in range(NG)
                        ]
                    elif have_h:
                        ops += [
                            ("hb", u, nb, nb)
                            for u in range(NU)
                            for nb in range(NB)
                        ]
                    if not x_first and x_lhsT is not None:
                        ops += [("x", nb, None, nb) for nb in range(NB)]
                    first_by_bank, last_by_bank = {}, {}
                    for i, op in enumerate(ops):
                        first_by_bank.setdefault(op[3], i)
                        last_by_bank[op[3]] = i
                    for i, (kind, idx, g, bank) in enumerate(ops):
                        start = i == first_by_bank[bank]
                        stop = i == last_by_bank[bank]
                        if kind == "x":
                            nc.tensor.matmul(
                                z[:, idx * 512 : (idx + 1) * 512],
                                x_lhsT,
                                Wk_sb[:, WW * w + idx * 512 : WW * w + (idx + 1) * 512],
                                start=start,
                                stop=stop,
                            )
                        elif kind == "h8":
                            nc.tensor.matmul(
                                z[:, g * 512 : (g + 1) * 512],
                                hT_prev[idx][:],
                                Wr_sb[:, idx, NG * w + g],
                                start=start,
                                stop=stop,
                                perf_mode=DR,
                            )
                        else:  # hb: bf16 chunk idx, bank g
                            nc.tensor.matmul(
                                z[:, g * 512 : (g + 1) * 512],
                                hTb_prev[:, idx, :],
                                Wrb_sb[:, idx, WW * w + g * 512 : WW * w + (g + 1) * 512],
                                start=start,
                                stop=stop,
                            )
                    # Gate math. Wave layout [i|f|o|g], each QW wide; PSUM
                    # holds SZ*z so every z-reading ACT applies scale 1/SZ.
                    # One merged sigmoid over the contiguous [i|f|o] block.
                    # sigmoid split at the PSUM bank boundary: [i|f] sits
                    # in bank 0, so tanh(g) — which gates the Pool-side t1 on
                    # the critical chain — issues right after the short
                    # bank-0 sigmoid instead of after a merged 768-col one.
                    sig = gpool.tile([128, 2 * QW], F32, tag="sig", name="sig")
                    tg = gpool.tile([128, QW], F32, tag="tg", name="tg")
                    sgo = gpool.tile([128, QW], F32, tag="sgo", name="sgo")
                    nc.scalar.activation(sig[:], z[:, 0 : 2 * QW], AF.Sigmoid, scale=1.0 / SZ)
                    nc.scalar.activation(tg[:], z[:, 3 * QW : 4 * QW], AF.Tanh, scale=1.0 / SZ)
                    nc.scalar.activation(sgo[:], z[:, 2 * QW : 3 * QW], AF.Sigmoid, scale=1.0 / SZ)
                    cs = c_sb[:, w * QW : (w + 1) * QW]
                    t1 = gpool.tile([128, QW], F32, tag="t1", name="t1")
                    # t1 = sig(i)*tanh(g) on Pool (mostly idle engine)
                    nc.gpsimd.tensor_mul(t1[:], sig[:, 0:QW], tg[:])
                    t2 = gpool.tile([128, QW], F32, tag="t2", name="t2")
                    nc.vector.tensor_mul(t2[:], sig[:, QW : 2 * QW], cs)
                    nc.vector.tensor_add(cs, t1[:], t2[:])
                    tcc = gpool.tile([128, QW], F32, tag="tcc", name="tcc")
                    nc.scalar.activation(tcc[:], cs, AF.Tanh)
                    if out_fmt == "fp8":
                        # h8 = (sig(o) * SH) * tanh(c), fp8, one fused DVE op
                        h8 = gpool.tile([128, QW], F8, tag="h8", name="h8")
                        nc.vector.scalar_tensor_tensor(
                            h8[:], sgo[:], SH, tcc[:],
                            ALU.mult, ALU.mult,
                        )
                        # u16-pair xbar transpose: word k of line m holds
                        # units (2k, 2k+1); deinterleave to the contiguous
                        # [K, 2, M] DoubleRow stationary layout (A = even
                        # units of the wave, B = odd).
                        hT16 = hpool.tile([128, QW // 2], U16, tag=f"hS{w}", name="hS")
                        nc.sync.dma_start_transpose(hT16[:], h8[:].bitcast(U16))
                        hTd = hpool.tile([128, 2, 128], F8, tag=f"hT{w}", name="hT")
                        hv = hT16[:].bitcast(F8).rearrange("k (m two) -> k two m", two=2)
                        nc.vector.tensor_copy(hTd[:, 0, :], hv[:, 0, :])
                        nc.vector.tensor_copy(hTd[:, 1, :], hv[:, 1, :])
                        hT_new[w] = hTd
                    else:
                        hbf = gpool.tile([128, QW], BF16, tag="hbf", name="hbf")
                        nc.vector.tensor_mul(hbf[:], sgo[:], tcc[:])
                        nch = QW // 128
                        nc.sync.dma_start_transpose(
                            hTb_new[:, w * nch : (w + 1) * nch, :], hbf[:]
                        )
                return (hT_new, hTb_new)

            def emit_dense(hTb_cur, out_idx, feedback):
                zp = zpool.tile([128, WW], F32, tag="z", name="zdense")
                zd = zp[0:I, 0:BC]
                for u in range(NU):
                    nc.tensor.matmul(
                        zd,
                        Wdb_sb[:, u, :],
                        hTb_cur[:, u, :],
                        start=(u == 0),
                        stop=(u == NU - 1),
                    )
                pf = gpool.tile([I, BC], F32, tag="pf", name="pf")
                nc.scalar.activation(pf[:], zd, AF.Identity, bias=bd_sb[:], scale=1.0 / SZ)
                nc.scalar.dma_start(outD[out_idx], pf[:])
                if not feedback:
                    return None
                pt = gpool.tile([KX, BC], BF16, tag="pT", name="pT")
                nc.gpsimd.memset(pt[I : I + 1, :], 1.0)
                nc.scalar.activation(pt[0:I, :], zd, AF.Identity, bias=bd_sb[:], scale=1.0 / SZ)
                return pt

            hT, hTb = None, None
            nblk_used = (n_warm + XBLK - 1) // XBLK
            xtiles = {}
            if nblk_used > 0:
                xtiles[0] = xpool.tile([KX, XBLK * BC], BF16, tag="xblk", name="xblk")
                nc.sync.dma_start(xtiles[0][:], xTbD[0])
            for t in range(n_warm):
                b = t // XBLK
                s = t % XBLK
                fmt = "fp8" if t < n_fp8 - 1 else "bf16"
                hT, hTb = emit_step(
                    xtiles[b][:, s * BC : (s + 1) * BC], hT, hTb,
                    x_first=True, out_fmt=fmt,
                )
                if fmt == "bf16":
                    hT = None
                if t % XBLK == 0 and b + 1 < nblk_used:
                    xtiles[b + 1] = xpool.tile([KX, XBLK * BC], BF16, tag="xblk", name="xblk")
                    nc.sync.dma_start(xtiles[b + 1][:], xTbD[b + 1])
                xtiles.pop(b - 1, None)
            pt = emit_dense(hTb, 0, feedback=(n_dec > 0))
            for d in range(n_dec):
                _, hTb = emit_step(
                    pt[:] if pt is not None else None, None, hTb,
                    x_first=False, out_fmt="bf16",
                )
                pt = emit_dense(hTb, d + 1, feedback=(d < n_dec - 1))

    nc.finalize()
    return nc


def prep_in_maps(inputs, Wk, Wr, b, Wd, bd, n_warm=T):
    """Host-side sharding + layout. inputs [B, T, I] fp32; returns 8 in_maps."""
    perm = _gate_perm()
    Wk_aug = np.concatenate(
        [np.asarray(Wk, np.float32), np.asarray(b, np.float32)[None, :]], axis=0
    )
    Wk_p = (Wk_aug[:, perm] * SZ).astype(NPBF16)               # [65, 4096], x256
    Wr_p = np.asarray(Wr, np.float32)[:, perm]                 # [1024, 4096]
    # Wr8[k, p, g, i, n] = SW * Wr_p[256p + 2k + i, 512g + n]
    Wr8 = (Wr_p * SW).astype(NPF8)
    Wr8 = Wr8.reshape(NW, 128, 2, 8, 512)                      # [p, k, i, g, n]
    Wr8 = Wr8.transpose(1, 0, 3, 2, 4).copy()                  # [k, p, g, i, n]
    # bf16 tail weights, same x256 z-scale as the fp8 path
    Wrb = (Wr_p * SZ).astype(NPBF16)
    Wrb = Wrb.reshape(NU, 128, 4 * U).transpose(1, 0, 2).copy()
    Wdb = (np.asarray(Wd, np.float32) * SZ).astype(NPBF16)
    Wdb = Wdb.reshape(NU, 128, I).transpose(1, 0, 2).copy()
    bd_c = np.asarray(bd, np.float32).reshape(I, 1).copy()

    x = np.asarray(inputs, np.float32)
    nblk = (n_warm + XBLK - 1) // XBLK
    in_maps = []
    for c in range(NCORES):
        xc = x[c * BC : (c + 1) * BC, :n_warm]                 # [BC, n_warm, I]
        xT = np.transpose(xc, (1, 2, 0))                       # [n_warm, I, BC]
        xTa = np.concatenate([xT, np.ones((n_warm, 1, BC), np.float32)], axis=1)
        if nblk * XBLK != n_warm:
            pad = np.zeros((nblk * XBLK - n_warm, KX, BC), np.float32)
            xTa = np.concatenate([xTa, pad], axis=0)
        xTb = (
            xTa.reshape(nblk, XBLK, KX, BC)
            .transpose(0, 2, 1, 3)
            .reshape(nblk, KX, XBLK * BC)
            .astype(NPBF16)
            .copy()
        )
        in_maps.append(
            {"xTb": xTb, "Wk": Wk_p, "Wr8": Wr8, "Wrb": Wrb, "Wdb": Wdb, "bdc": bd_c}
        )
    return in_maps


_NC_CACHE = {}


def _get_nc(n_warm, n_dec):
    key = (n_warm, n_dec)
    if key not in _NC_CACHE:
        _NC_CACHE[key] = build_nc(n_warm, n_dec)
    return _NC_CACHE[key]


def run(inputs, Wk, Wr, b, Wd, bd, n_warm, n_dec, trace=False):
    nc = _get_nc(n_warm, n_dec)
    in_maps = prep_in_maps(inputs, Wk, Wr, b, Wd, bd, n_warm)
    res = run_bass_kernel_spmd(nc, in_maps, list(range(NCORES)), trace=trace)
    outs = [np.asarray(res.results[c]["out"], np.float32) for c in range(NCORES)]
    # out[c]: [n_dec+1, I, BC] -> preds [B, n_dec+1, I]
    preds = np.concatenate([o.transpose(2, 0, 1) for o in outs], axis=0)
    return preds, res


def kernel(inputs, Wk, Wr, b, Wd, bd, output_indices, output_steps):
    n_dec = int(output_steps) - 1
    preds, _ = run(inputs, Wk, Wr, b, Wd, bd, T, n_dec)
    idx = np.asarray(output_indices, np.int64)
    return np.take(preds, idx, axis=-1).astype(np.float32)


# revision 5
# speedup vs baseline: 1.0983x; 1.0326x over previous
"""Trainium2 Bass kernel for the autoregressive LSTM — fp8 DoubleRow body
with a bf16 tail.

Math (Keras LSTMCell, gate order i,f,g,o in the reference):
    z = x @ Wk + h @ Wr + b
    c = sigmoid(f)*c + sigmoid(i)*tanh(g)
    h = sigmoid(o)*tanh(c)
Warmup over T=256 input steps, then S=64 autoregressive decode steps through
a dense head p = h @ Wd + bd fed back as the next input.

Sharding: pure data-parallel over batch, 1024/8 = 128 examples per core.

Precision schedule: the first T-NTAIL warmup steps run with h cast to
fp8e4m3 (x8) and Wr prepacked fp8 (x32) in DoubleRow pair layout — one
DoubleRow matmul contracts TWO 128-row unit chunks. LSTM forget-gate damping
(~0.5/step) erases that quantization noise within ~20 steps, so the last
NTAIL warmup steps and the whole decode run in bf16 (baseline numerics);
outputs match the bf16 kernel. PSUM holds 256*z uniformly in both modes
(Wk, bf16-Wr, bf16-Wd all pre-scaled x256); gate ACTs apply scale 1/256.

fp8 step epilogue: h8 = (sig(o)*8)*tanh(c) via one fused DVE op, then a
u16-pair xbar transpose + engine-alternating deinterleave copies produce the
contiguous [K, 2, M] DoubleRow stationary tile. bf16 step epilogue: hbf =
sig(o)*tanh(c) bf16 + merged xbar transpose into chunk-major [128, NU, 128].
Transposes alternate between the SP and ACT DMA queues so the 4 per step run
on two rings.
"""

import sys

sys.path.insert(0, "/opt/trn_rl_repo")

import numpy as np

import concourse.bass as bass
import concourse.bacc as bacc
import concourse.mybir as mybir
from concourse.tile import TileContext
from concourse.bass_utils import run_bass_kernel_spmd

F32 = mybir.dt.float32
BF16 = mybir.dt.bfloat16
F8 = mybir.dt.float8e4
U16 = mybir.dt.uint16
NPBF16 = mybir.dt.np(mybir.dt.bfloat16)
NPF8 = mybir.dt.np(mybir.dt.float8e4)
AF = mybir.ActivationFunctionType
ALU = mybir.AluOpType
DR = mybir.MatmulPerfMode.DoubleRow

B, T, I, U, S = 1024, 256, 64, 1024, 64
NCORES = 8
BC = B // NCORES          # 128 batch per core
KX = I + 1                # x rows + ones row for folded bias
XBLK = 4                  # warmup steps per input-stream DMA block
NU = U // 128             # 8 recurrent k-chunks (bf16 path)

NW = 4                    # waves per step (each covers U/NW units, 4U/NW z-cols)
QW = U // NW              # units per wave (256)
WW = 4 * QW               # z columns per wave (1024)
NB = WW // 512            # 512-col bank slices per wave (2)
NG = NB                   # 512-col DR groups per wave (2): one per bank

SH = 8.0                  # h scale into fp8
SW = 32.0                 # weight scale into fp8
SZ = SH * SW              # PSUM holds SZ * z in BOTH modes
NTAIL = 8                 # final warmup steps in bf16 (decode is always bf16)


def _gate_perm():
    """Column permutation: reference gate order [i|f|g|o] (1024 each) ->
    NW waves of [i_q | f_q | o_q | g_q] (QW each)."""
    i0, f0, g0, o0 = 0, U, 2 * U, 3 * U
    parts = []
    for w in range(NW):
        for g in (i0, f0, o0, g0):
            parts.append(np.arange(QW) + g + w * QW)
    return np.concatenate(parts)


def build_nc(n_warm=T, n_dec=S - 1, n_tail=NTAIL, for_sim=False):
    nc = bacc.Bacc(None, target_bir_lowering=False, debug=True) if for_sim else bacc.Bacc()
    n_fp8 = max(0, n_warm - n_tail)

    nblk = (n_warm + XBLK - 1) // XBLK
    xTbD = nc.declare_dram_parameter("xTb", [nblk, KX, XBLK * BC], BF16, isOutput=False)
    WkD = nc.declare_dram_parameter("Wk", [KX, 4 * U], BF16, isOutput=False)
    # Wr8[k, p, g, i, n] = SW * Wr_perm[256p + 2k + i, 512g + n]
    WrD = nc.declare_dram_parameter("Wr8", [128, NW, 8, 2, 512], F8, isOutput=False)
    WrbD = nc.declare_dram_parameter("Wrb", [128, NU, 4 * U], BF16, isOutput=False)
    WdbD = nc.declare_dram_parameter("Wdb", [128, NU, I], BF16, isOutput=False)
    bdD = nc.declare_dram_parameter("bdc", [I, 1], F32, isOutput=False)
    outD = nc.declare_dram_parameter("out", [n_dec + 1, I, BC], F32, isOutput=True)

    with TileContext(nc) as tc:
        with (
            tc.tile_pool(name="const", bufs=1) as cpool,
            tc.tile_pool(name="xp", bufs=2) as xpool,
            tc.tile_pool(name="state", bufs=2) as hpool,
            tc.tile_pool(name="gates", bufs=2) as gpool,
            tc.tile_pool(name="psum", bufs=3, space="PSUM") as zpool,
        ):
            Wk_sb = cpool.tile([KX, 4 * U], BF16)
            Wr_sb = cpool.tile([128, NW, 8, 2, 512], F8)
            Wrb_sb = cpool.tile([128, NU, 4 * U], BF16)
            Wdb_sb = cpool.tile([128, NU, I], BF16)
            bd_sb = cpool.tile([I, 1], F32)
            c_sb = cpool.tile([128, U], F32)
            nc.sync.dma_start(Wk_sb[:], WkD[:])
            if n_fp8 > 0:
                nc.sync.dma_start(Wr_sb[:], WrD[:])
            nc.sync.dma_start(Wrb_sb[:], WrbD[:])
            nc.sync.dma_start(Wdb_sb[:], WdbD[:])
            nc.sync.dma_start(bd_sb[:], bdD[:])
            nc.gpsimd.memset(c_sb[:], 0.0)

            def emit_step(x_lhsT, hT_prev, hTb_prev, x_first, out_fmt):
                """One LSTM step. Consumes EITHER hT_prev (fp8 pair tiles)
                or hTb_prev (bf16 chunk-major tile). Produces hT (list) or
                hTb per out_fmt."""
                use_fp8 = hT_prev is not None
                have_h = use_fp8 or hTb_prev is not None
                hT_new = [None] * NW
                hTb_new = (
                    hpool.tile([128, NU, 128], BF16, tag="hTb", name="hTb")
                    if out_fmt == "bf16"
                    else None
                )
                for w in range(NW):
                    z = zpool.tile([128, WW], F32, tag="z", name="z")
                    # Ordered op list; PSUM zero regions are bank-granular
                    # (2KB): exactly one start (first writer) and one stop
                    # (last writer) per bank.
                    ops = []  # (kind, idx, g, bank)
                    if x_first and x_lhsT is not None:
                        ops += [("x", nb, None, nb) for nb in range(NB)]
                    if use_fp8:
                        ops += [
                            ("h8", p, g, g)
                            for p in range(NW)
                            for g in range(NG)
                        ]
                    elif have_h:
                        ops += [
                            ("hb", u, nb, nb)
                            for u in range(NU)
                            for nb in range(NB)
                        ]
                    if not x_first and x_lhsT is not None:
                        ops += [("x", nb, None, nb) for nb in range(NB)]
                    first_by_bank, last_by_bank = {}, {}
                    for i, op in enumerate(ops):
                        first_by_bank.setdefault(op[3], i)
                        last_by_bank[op[3]] = i
                    for i, (kind, idx, g, bank) in enumerate(ops):
                        start = i == first_by_bank[bank]
                        stop = i == last_by_bank[bank]
                        if kind == "x":
                            nc.tensor.matmul(
                                z[:, idx * 512 : (idx + 1) * 512],
                                x_lhsT,
                                Wk_sb[:, WW * w + idx * 512 : WW * w + (idx + 1) * 512],
                                start=start,
                                stop=stop,
                            )
                        elif kind == "h8":
                            nc.tensor.matmul(
                                z[:, g * 512 : (g + 1) * 512],
                                hT_prev[idx][:],
                                Wr_sb[:, idx, NG * w + g],
                                start=start,
                                stop=stop,
                                perf_mode=DR,
                            )
                        else:  # hb: bf16 chunk idx, bank g
                            nc.tensor.matmul(
                                z[:, g * 512 : (g + 1) * 512],
                                hTb_prev[:, idx, :],
                                Wrb_sb[:, idx, WW * w + g * 512 : WW * w + (g + 1) * 512],
                                start=start,
                                stop=stop,
                            )
                    # Gate math. Wave layout [i|f|o|g], each QW wide; PSUM
                    # holds SZ*z so every z-reading ACT applies scale 1/SZ.
                    # One merged sigmoid over the contiguous [i|f|o] block.
                    # sigmoid split at the PSUM bank boundary: [i|f] sits
                    # in bank 0, so tanh(g) — which gates the Pool-side t1 on
                    # the critical chain — issues right after the short
                    # bank-0 sigmoid instead of after a merged 768-col one.
                    sig = gpool.tile([128, 2 * QW], F32, tag="sig", name="sig")
                    tg = gpool.tile([128, QW], F32, tag="tg", name="tg")
                    sgo = gpool.tile([128, QW], F32, tag="sgo", name="sgo")
                    nc.scalar.activation(sig[:], z[:, 0 : 2 * QW], AF.Sigmoid, scale=1.0 / SZ)
                    nc.scalar.activation(tg[:], z[:, 3 * QW : 4 * QW], AF.Tanh, scale=1.0 / SZ)
                    nc.scalar.activation(sgo[:], z[:, 2 * QW : 3 * QW], AF.Sigmoid, scale=1.0 / SZ)
                    cs = c_sb[:, w * QW : (w + 1) * QW]
                    t1 = gpool.tile([128, QW], F32, tag="t1", name="t1")
                    # t1 = sig(i)*tanh(g) on DVE (fast engine takes the
                    # late, tg-gated product); t2 = sig(f)*c on Pool (its
                    # input sig(f) is ready earliest)
                    nc.vector.tensor_mul(t1[:], sig[:, 0:QW], tg[:])
                    t2 = gpool.tile([128, QW], F32, tag="t2", name="t2")
                    nc.gpsimd.tensor_mul(t2[:], sig[:, QW : 2 * QW], cs)
                    nc.vector.tensor_add(cs, t1[:], t2[:])
                    tcc = gpool.tile([128, QW], F32, tag="tcc", name="tcc")
                    nc.scalar.activation(tcc[:], cs, AF.Tanh)
                    if out_fmt == "fp8":
                        # h8 = (sig(o) * SH) * tanh(c), fp8, one fused DVE op
                        h8 = gpool.tile([128, QW], F8, tag="h8", name="h8")
                        nc.vector.scalar_tensor_tensor(
                            h8[:], sgo[:], SH, tcc[:],
                            ALU.mult, ALU.mult,
                        )
                        # u16-pair xbar transpose: word k of line m holds
                        # units (2k, 2k+1); deinterleave to the contiguous
                        # [K, 2, M] DoubleRow stationary layout (A = even
                        # units of the wave, B = odd).
                        hT16 = hpool.tile([128, QW // 2], U16, tag=f"hS{w}", name="hS")
                        nc.sync.dma_start_transpose(hT16[:], h8[:].bitcast(U16))
                        hTd = hpool.tile([128, 2, 128], F8, tag=f"hT{w}", name="hT")
                        hv = hT16[:].bitcast(F8).rearrange("k (m two) -> k two m", two=2)
                        nc.vector.tensor_copy(hTd[:, 0, :], hv[:, 0, :])
                        nc.vector.tensor_copy(hTd[:, 1, :], hv[:, 1, :])
                        hT_new[w] = hTd
                    else:
                        hbf = gpool.tile([128, QW], BF16, tag="hbf", name="hbf")
                        nc.vector.tensor_mul(hbf[:], sgo[:], tcc[:])
                        nch = QW // 128
                        nc.sync.dma_start_transpose(
                            hTb_new[:, w * nch : (w + 1) * nch, :], hbf[:]
                        )
                return (hT_new, hTb_new)

            def emit_dense(hTb_cur, out_idx, feedback):
                zp = zpool.tile([128, WW], F32, tag="z", name="zdense")
                zd = zp[0:I, 0:BC]
                for u in range(NU):
                    nc.tensor.matmul(
                        zd,
                        Wdb_sb[:, u, :],
                        hTb_cur[:, u, :],
                        start=(u == 0),
                        stop=(u == NU - 1),
                    )
                pf = gpool.tile([I, BC], F32, tag="pf", name="pf")
                nc.scalar.activation(pf[:], zd, AF.Identity, bias=bd_sb[:], scale=1.0 / SZ)
                nc.scalar.dma_start(outD[out_idx], pf[:])
                if not feedback:
                    return None
                pt = gpool.tile([KX, BC], BF16, tag="pT", name="pT")
                nc.gpsimd.memset(pt[I : I + 1, :], 1.0)
                nc.scalar.activation(pt[0:I, :], zd, AF.Identity, bias=bd_sb[:], scale=1.0 / SZ)
                return pt

            hT, hTb = None, None
            nblk_used = (n_warm + XBLK - 1) // XBLK
            xtiles = {}
            if nblk_used > 0:
                xtiles[0] = xpool.tile([KX, XBLK * BC], BF16, tag="xblk", name="xblk")
                nc.sync.dma_start(xtiles[0][:], xTbD[0])
            for t in range(n_warm):
                b = t // XBLK
                s = t % XBLK
                fmt = "fp8" if t < n_fp8 - 1 else "bf16"
                hT, hTb = emit_step(
                    xtiles[b][:, s * BC : (s + 1) * BC], hT, hTb,
                    x_first=True, out_fmt=fmt,
                )
                if fmt == "bf16":
                    hT = None
                if t % XBLK == 0 and b + 1 < nblk_used:
                    xtiles[b + 1] = xpool.tile([KX, XBLK * BC], BF16, tag="xblk", name="xblk")
                    nc.sync.dma_start(xtiles[b + 1][:], xTbD[b + 1])
                xtiles.pop(b - 1, None)
            pt = emit_dense(hTb, 0, feedback=(n_dec > 0))
            for d in range(n_dec):
                _, hTb = emit_step(
                    pt[:] if pt is not None else None, None, hTb,
                    x_first=False, out_fmt="bf16",
                )
                pt = emit_dense(hTb, d + 1, feedback=(d < n_dec - 1))

    nc.finalize()
    return nc


def prep_in_maps(inputs, Wk, Wr, b, Wd, bd, n_warm=T):
    """Host-side sharding + layout. inputs [B, T, I] fp32; returns 8 in_maps."""
    perm = _gate_perm()
    Wk_aug = np.concatenate(
        [np.asarray(Wk, np.float32), np.asarray(b, np.float32)[None, :]], axis=0
    )
    Wk_p = (Wk_aug[:, perm] * SZ).astype(NPBF16)               # [65, 4096], x256
    Wr_p = np.asarray(Wr, np.float32)[:, perm]                 # [1024, 4096]
    # Wr8[k, p, g, i, n] = SW * Wr_p[256p + 2k + i, 512g + n]
    Wr8 = (Wr_p * SW).astype(NPF8)
    Wr8 = Wr8.reshape(NW, 128, 2, 8, 512)                      # [p, k, i, g, n]
    Wr8 = Wr8.transpose(1, 0, 3, 2, 4).copy()                  # [k, p, g, i, n]
    # bf16 tail weights, same x256 z-scale as the fp8 path
    Wrb = (Wr_p * SZ).astype(NPBF16)
    Wrb = Wrb.reshape(NU, 128, 4 * U).transpose(1, 0, 2).copy()
    Wdb = (np.asarray(Wd, np.float32) * SZ).astype(NPBF16)
    Wdb = Wdb.reshape(NU, 128, I).transpose(1, 0, 2).copy()
    bd_c = np.asarray(bd, np.float32).reshape(I, 1).copy()

    x = np.asarray(inputs, np.float32)
    nblk = (n_warm + XBLK - 1) // XBLK
    in_maps = []
    for c in range(NCORES):
        xc = x[c * BC : (c + 1) * BC, :n_warm]                 # [BC, n_warm, I]
        xT = np.transpose(xc, (1, 2, 0))                       # [n_warm, I, BC]
        xTa = np.concatenate([xT, np.ones((n_warm, 1, BC), np.float32)], axis=1)
        if nblk * XBLK != n_warm:
            pad = np.zeros((nblk * XBLK - n_warm, KX, BC), np.float32)
            xTa = np.concatenate([xTa, pad], axis=0)
        xTb = (
            xTa.reshape(nblk, XBLK, KX, BC)
            .transpose(0, 2, 1, 3)
            .reshape(nblk, KX, XBLK * BC)
            .astype(NPBF16)
            .copy()
        )
        in_maps.append(
            {"xTb": xTb, "Wk": Wk_p, "Wr8": Wr8, "Wrb": Wrb, "Wdb": Wdb, "bdc": bd_c}
        )
    return in_maps


_NC_CACHE = {}


def _get_nc(n_warm, n_dec):
    key = (n_warm, n_dec)
    if key not in _NC_CACHE:
        _NC_CACHE[key] = build_nc(n_warm, n_dec)
    return _NC_CACHE[key]


def run(inputs, Wk, Wr, b, Wd, bd, n_warm, n_dec, trace=False):
    nc = _get_nc(n_warm, n_dec)
    in_maps = prep_in_maps(inputs, Wk, Wr, b, Wd, bd, n_warm)
    res = run_bass_kernel_spmd(nc, in_maps, list(range(NCORES)), trace=trace)
    outs = [np.asarray(res.results[c]["out"], np.float32) for c in range(NCORES)]
    # out[c]: [n_dec+1, I, BC] -> preds [B, n_dec+1, I]
    preds = np.concatenate([o.transpose(2, 0, 1) for o in outs], axis=0)
    return preds, res


def kernel(inputs, Wk, Wr, b, Wd, bd, output_indices, output_steps):
    n_dec = int(output_steps) - 1
    preds, _ = run(inputs, Wk, Wr, b, Wd, bd, T, n_dec)
    idx = np.asarray(output_indices, np.int64)
    return np.take(preds, idx, axis=-1).astype(np.float32)
